# revision 19
# baseline (speedup 1.0000x reference)
"""Trainium2 Bass kernel for nn_DeltaRuleMemory (decayed causal linear attention
with RoPE, ternary-STE k/v quantization and beta key gating).

Sharding: 8 cores = batch (2) x head-groups (4 groups of 4 heads). Each core
computes its (b, head-group) slice end-to-end; the only cross-core exchange is
a pair of 1-float AllReduces for the global ternary-quantization thresholds.
Host sums the 4 per-head-group partial output projections per batch.

Algorithm note: decay alpha = sigmoid(alpha_log) < 0.5 for every head, so
exp(log_alpha * d) underflows to exactly 0.0f for d >= 128 (as it does in the
reference's T x T decay matrix). Attention is therefore computed exactly as a
banded product: each 128-query chunk attends to its own chunk (masked decay)
and the previous chunk only; the two contributions accumulate in PSUM.

Precision: the ternary threshold compare is sensitive to k/v values, so the
k/v projections run as a 3-chain fp16 hi/lo split (x = xh + xl, W = Wh + Wl;
k = xh@Wh + xl@Wh + xh@Wl, fp32 PSUM accumulation) which is fp32-accurate to
~2^-21 but runs at bf16 matmul speed. q/beta run a single fp16 hi chain and
the attention + output projection run in fp16 (~1e-4 relative, well below the
quantization-boundary noise).

Schedule: phase A1 projects k and q and derives thr_k, so ternarized k, the
score matmuls and the decay multiply all execute under the shadow of phase A3
(the v projection). Only thr_v, v-ternarize, AV and the output projection sit
on the tail. Ternarize uses sign(x-thr)+sign(x+thr) pairs on Activation
(values {-2,0,2}; dt2 and a halved beta carry the 1/2 compensations), with
half the v blocks on GpSimd via is_gt/is_lt to balance engines.
"""
import numpy as np
from contextlib import ExitStack

import concourse.bass as bass
import concourse.tile as tile
import concourse.mybir as mybir
from concourse import bacc
from concourse.bass import ds
from concourse.bass_utils import run_bass_kernel_spmd

F32 = mybir.dt.float32
F16 = mybir.dt.float16
MUL = mybir.AluOpType.mult
ADD = mybir.AluOpType.add
SUB = mybir.AluOpType.subtract
SIGN = mybir.ActivationFunctionType.Sign

B, D_MODEL, NH, HD = 2, 1024, 16, 64
INNER = NH * HD
N_CORES = 8
HG = 4              # heads per core
GD = HG * HD        # inner dims per core (256)
C = 128             # attention chunk
ROPE_BASE = 10000.0
THR_MIN, THR_MAX = 0.01, 10.0

_NC_CACHE = {}


def build_nc(T=2048, n_cores=N_CORES, use_cc=True, repeat=1):
    """Build the SPMD bass program (identical on every core)."""
    KT = D_MODEL // 128          # 8 contraction tiles
    NCH = T // C                 # chunks
    W5 = min(512, T)             # free-dim window for [*, T] processing
    NW = T // W5

    nc = bacc.Bacc("TRN2", target_bir_lowering=False, debug=False,
                   enable_asserts=True, num_devices=n_cores)

    xh_d = nc.dram_tensor("xh", [D_MODEL, T], F16, kind="ExternalInput").ap()
    xl_d = nc.dram_tensor("xl", [D_MODEL, T], F16, kind="ExternalInput").ap()
    wkh_d = nc.dram_tensor("wkh", [D_MODEL, GD], F16, kind="ExternalInput").ap()
    wkl_d = nc.dram_tensor("wkl", [D_MODEL, GD], F16, kind="ExternalInput").ap()
    wvh_d = nc.dram_tensor("wvh", [D_MODEL, GD], F16, kind="ExternalInput").ap()
    wvl_d = nc.dram_tensor("wvl", [D_MODEL, GD], F16, kind="ExternalInput").ap()
    wq_d = nc.dram_tensor("wq", [D_MODEL, GD], F16, kind="ExternalInput").ap()
    wb_d = nc.dram_tensor("wb", [D_MODEL, HG], F16, kind="ExternalInput").ap()
    bbx_d = nc.dram_tensor("bbx", [128, HG], F32, kind="ExternalInput").ap()
    wo_d = nc.dram_tensor("wo", [GD, D_MODEL], F16, kind="ExternalInput").ap()
    ct_d = nc.dram_tensor("ct", [128, T], F32, kind="ExternalInput").ap()
    st_d = nc.dram_tensor("st", [128, T], F32, kind="ExternalInput").ap()
    dt2_d = nc.dram_tensor("dt2", [128, HG, 2 * C], F32, kind="ExternalInput").ap()
    out_d = nc.dram_tensor("out", [T, D_MODEL], F16, kind="ExternalOutput").ap()

    if use_cc:
        cck_in = nc.dram_tensor("cck_in", [1, 1], F32)
        cck_out = nc.dram_tensor("cck_out", [1, 1], F32, addr_space="Shared")
        ccv_in = nc.dram_tensor("ccv_in", [1, 1], F32)
        ccv_out = nc.dram_tensor("ccv_out", [1, 1], F32, addr_space="Shared")

    n_elem = float(B * T * INNER) if use_cc else float(T * GD)

    with tile.TileContext(nc) as tc, ExitStack() as ctx:
        cpool = ctx.enter_context(tc.tile_pool(name="const", bufs=1))
        wpool = ctx.enter_context(tc.tile_pool(name="w", bufs=2))
        big = ctx.enter_context(tc.tile_pool(name="big", bufs=1))
        scr = ctx.enter_context(tc.tile_pool(name="scr", bufs=3))
        gl = ctx.enter_context(tc.tile_pool(name="gl", bufs=2))
        tiny = ctx.enter_context(tc.tile_pool(name="tiny", bufs=1))
        xpool = ctx.enter_context(tc.tile_pool(name="xs", bufs=2))

        # ---- weights for the k chain + first x window go first so the PE
        # starts early; everything else queues behind them ----
        wkh_sb = cpool.tile([128, KT, GD], F16, tag="wkh")
        nc.sync.dma_start(wkh_sb[:], wkh_d.rearrange("(ko p) m -> p ko m", p=128))
        wkl_sb = cpool.tile([128, KT, GD], F16, tag="wkl")
        nc.sync.dma_start(wkl_sb[:], wkl_d.rearrange("(ko p) m -> p ko m", p=128))

        xh_r = xh_d.rearrange("(ko p) t -> p ko t", p=128)
        xl_r = xl_d.rearrange("(ko p) t -> p ko t", p=128)

        def load_x(w, name_sfx, with_lo=True, nsplit=1):
            win = ds(w * W5, W5)
            xhw = xpool.tile([128, KT, W5], F16, tag="xh", name=f"xh{name_sfx}")
            xlw = xpool.tile([128, KT, W5], F16, tag="xl", name=f"xl{name_sfx}")
            step = KT // nsplit
            for s in range(nsplit):
                ks = ds(s * step, step)
                nc.sync.dma_start(xhw[:, ks, :], xh_r[:, ks, win])
            if with_lo:
                for s in range(nsplit):
                    ks = ds(s * step, step)
                    nc.sync.dma_start(xlw[:, ks, :], xl_r[:, ks, win])
            return xhw, xlw

        xw_pend = load_x(0, "_a0", nsplit=2)

        wq_sb = cpool.tile([128, KT, GD], F16, tag="wq")
        nc.sync.dma_start(wq_sb[:], wq_d.rearrange("(ko p) m -> p ko m", p=128))
        ct_sb = cpool.tile([128, T], F32, tag="ct")
        st_sb = cpool.tile([128, T], F32, tag="st")
        nc.sync.dma_start(ct_sb[:], ct_d[:])
        nc.sync.dma_start(st_sb[:], st_d[:])
        dt2_sb = cpool.tile([128, HG, 2 * C], F32, tag="dt2")
        nc.sync.dma_start(dt2_sb[:], dt2_d[:])
        wvh_sb = cpool.tile([128, KT, GD], F16, tag="wvh")
        nc.sync.dma_start(wvh_sb[:], wvh_d.rearrange("(ko p) m -> p ko m", p=128))
        wvl_sb = cpool.tile([128, KT, GD], F16, tag="wvl")
        nc.sync.dma_start(wvl_sb[:], wvl_d.rearrange("(ko p) m -> p ko m", p=128))
        wb_sb = cpool.tile([128, KT, HG], F16, tag="wb")
        nc.sync.dma_start(wb_sb[:], wb_d.rearrange("(ko p) h -> p ko h", p=128))
        bbx_sb = cpool.tile([128, HG], F32, tag="bbx")
        nc.sync.dma_start(bbx_sb[:], bbx_d[:])

        for rep in range(repeat):
            sfx = f"_r{rep}" if repeat > 1 else ""

            kT = [big.tile([128, T], F32, tag=f"kT{i}", name=f"kT{i}{sfx}") for i in range(2)]
            kTt = [big.tile([128, T], F16, tag=f"kTt{i}", name=f"kTt{i}{sfx}") for i in range(2)]
            qT = [big.tile([128, T], F16, tag=f"qT{i}", name=f"qT{i}{sfx}") for i in range(2)]
            v_sb = big.tile([128, NCH, GD], F32, tag="v", name=f"v{sfx}")
            # zero-padded ternary v: head h occupies columns (h%2)*64..+64 of
            # slot [jc, h]; the other half stays 0 so AV matmuls write the full
            # 128-row PSUM with head pairs at partition offsets 0/64.
            vbt = big.tile([128, NCH, HG, 2 * HD], F16, tag="vbt", name=f"vbt{sfx}")
            blog = big.tile([128, NCH, HG], F32, tag="blog", name=f"blog{sfx}")
            beta = big.tile([128, NCH, HG], F16, tag="beta", name=f"beta{sfx}")
            beta_h = big.tile([128, NCH, HG], F16, tag="beta_h", name=f"beta_h{sfx}")
            oT = big.tile([128, 2, T], F16, tag="oT", name=f"oT{sfx}")
            sts_all = big.tile([128, NCH, HG, 2 * C], F16, tag="sts", name=f"sts{sfx}")
            acc = tiny.tile([128, 16], F32, tag="acc", name="acc" + sfx)

            # zero the dead halves of vbt once, off the critical path
            vbt4 = vbt[:].rearrange("p c h (u d) -> p c h u d", u=2)
            nc.vector.memset(vbt4[:, :, 0::2, 1, :], 0.0)
            nc.vector.memset(vbt4[:, :, 1::2, 0, :], 0.0)

            def rope(ps, dst, win, nm):
                """dst[:, win] = rope(ps); rotate copies split Act/Pool."""
                rot = scr.tile([128, W5], F32, tag="rot", name=f"rot{nm}")
                nc.scalar.copy(rot[ds(0, 32), :], ps[ds(32, 32), :])
                nc.vector.tensor_copy(rot[ds(32, 32), :], ps[ds(0, 32), :])
                nc.scalar.copy(rot[ds(64, 32), :], ps[ds(96, 32), :])
                nc.vector.tensor_copy(rot[ds(96, 32), :], ps[ds(64, 32), :])
                nc.vector.tensor_tensor(rot[:], rot[:], st_sb[:, win], MUL)
                nc.vector.tensor_tensor(dst[:, win], ps[:], ct_sb[:, win], MUL)
                nc.vector.tensor_tensor(dst[:, win], dst[:, win], rot[:], ADD)

            def thr_chain(col_lo, col_hi, cc_pair, pool, tag, nm):
                """acc cols [col_lo, col_hi) -> broadcast [128,2] (thr, -thr)."""
                onesP = tiny.tile([128, 1], F32, tag="onesP", name=f"onesP{nm}")
                nc.vector.memset(onesP[:], 1.0)
                pstt = pool.tile([1, col_hi - col_lo], F32, tag=tag, name=f"pst{nm}")
                nc.tensor.matmul(pstt[:], onesP[:], acc[:, ds(col_lo, col_hi - col_lo)],
                                 start=True, stop=True)
                sct = tiny.tile([1, 1], F32, tag=f"sc{nm}", name=f"sc{nm}")
                nc.vector.tensor_reduce(sct[:], pstt[0:1, :],
                                        axis=mybir.AxisListType.X, op=ADD)
                if cc_pair is not None:
                    cin, cout = cc_pair
                    nc.sync.dma_start(cin[:], sct[:])
                    nc.gpsimd.collective_compute(
                        "AllReduce", ADD,
                        replica_groups=[list(range(n_cores))],
                        ins=[cin[:]], outs=[cout[:]])
                    tott = tiny.tile([1, 1], F32, tag=f"tot{nm}", name=f"tot{nm}")
                    nc.sync.dma_start(tott[:], cout[:])
                else:
                    tott = sct
                th = tiny.tile([1, 2], F32, tag=f"th{nm}", name=f"th{nm}")
                nc.vector.tensor_scalar(th[0:1, 0:1], tott[0:1, :], 1.0 / n_elem, None, MUL)
                nc.vector.tensor_scalar(th[0:1, 0:1], th[0:1, 0:1], THR_MIN, THR_MAX,
                                        mybir.AluOpType.max, mybir.AluOpType.min)
                nc.vector.tensor_scalar(th[0:1, 1:2], th[0:1, 0:1], -1.0, None, MUL)
                ones1 = tiny.tile([1, 128], F32, tag="ones1", name=f"ones1{nm}")
                nc.vector.memset(ones1[:], 1.0)
                psb2 = pool.tile([128, 2], F32, tag=tag, name=f"psb2{nm}")
                nc.tensor.matmul(psb2[:], ones1[:], th[:], start=True, stop=True)
                thb = tiny.tile([128, 2], F32, tag=f"thb{nm}", name=f"thb{nm}")
                nc.vector.tensor_copy(thb[:], psb2[:])   # [thr, -thr]
                return thb

            with tc.tile_pool(name="ppa" + sfx, bufs=1, space="PSUM") as ppa:
                # ---- phase A1: k (3-chain) + q (hi chain) projections ----
                with tc.tile_pool(name="ppq" + sfx, bufs=2, space="PSUM") as ppq:
                    for w in range(NW):
                        win = ds(w * W5, W5)
                        xhw, xlw = xw_pend
                        if w + 1 < NW:
                            xw_pend = load_x(w + 1, f"_a{w + 1}{sfx}")
                        for mt in range(2):
                            ps = ppq.tile([128, W5], F32, tag="proj")
                            chains = ((wkh_sb, xhw), (wkh_sb, xlw), (wkl_sb, xhw))
                            n = len(chains) * KT
                            i = 0
                            for wt_, xt_ in chains:
                                for kt_i in range(KT):
                                    nc.tensor.matmul(ps[:], wt_[:, kt_i, ds(mt * 128, 128)],
                                                     xt_[:, kt_i, :],
                                                     start=(i == 0), stop=(i == n - 1))
                                    i += 1
                            rope(ps, kT[mt], win, f"k{mt}_{w}{sfx}")
                            nc.vector.tensor_reduce(acc[:, ds(w * 2 + mt, 1)], kT[mt][:, win],
                                                    axis=mybir.AxisListType.X, op=ADD,
                                                    apply_absolute_value=True)
                        for mt in range(2):
                            psq = ppq.tile([128, W5], F32, tag="proj")
                            for kt_i in range(KT):
                                nc.tensor.matmul(psq[:], wq_sb[:, kt_i, ds(mt * 128, 128)],
                                                 xhw[:, kt_i, :],
                                                 start=(kt_i == 0), stop=(kt_i == KT - 1))
                            rope(psq, qT[mt], win, f"q{mt}_{w}{sfx}")

                    # ---- thr_k; ternarize k; scores (hide under phase A3) ----
                    thbk = thr_chain(0, 8, (cck_in, cck_out) if use_cc else None,
                                     ppq, "pt", "k" + sfx)
                    for w in range(NW):
                        win = ds(w * W5, W5)
                        for mt in range(2):
                            a1 = scr.tile([128, W5], F16, tag="rot", name=f"a1k{mt}_{w}{sfx}")
                            a2 = scr.tile([128, W5], F16, tag="rot", name=f"a2k{mt}_{w}{sfx}")
                            nc.scalar.activation(a1[:], kT[mt][:, win], SIGN, bias=thbk[:, 1:2])
                            nc.scalar.activation(a2[:], kT[mt][:, win], SIGN, bias=thbk[:, 0:1])
                            nc.vector.tensor_tensor(kTt[mt][:, win], a1[:], a2[:], ADD)
                    for jc in range(NCH):
                        ilen = min(2 * C, T - jc * C)
                        for grp in range(2):
                            spg = ppa.tile([128, 2, 2 * C], F32, tag=f"s{grp}",
                                           name=f"s{grp}_{jc}{sfx}")
                            for j, h in enumerate((grp, grp + 2)):
                                tl, po = h // 2, (h % 2) * 64
                                nc.tensor.matmul(
                                    spg[:, j, 0:ilen],
                                    kTt[tl][ds(po, 64), ds(jc * C, C)],
                                    qT[tl][ds(po, 64), ds(jc * C, ilen)],
                                    start=True, stop=True)
                            nc.vector.tensor_tensor(
                                sts_all[:, jc, ds(grp * 2, 2), 0:ilen],
                                spg[:, :, 0:ilen],
                                dt2_sb[:, ds(grp * 2, 2), 0:ilen], MUL)

                # ---- phase A3: v (3-chain) + beta projections ----
                with tc.tile_pool(name="ppv" + sfx, bufs=2, space="PSUM") as ppv:
                    xw_pend = load_x(0, "_b0" + sfx)
                    for w in range(NW):
                        xhw, xlw = xw_pend
                        if w + 1 < NW:
                            xw_pend = load_x(w + 1, f"_b{w + 1}{sfx}")
                        for sub in range(W5 // C):
                            tt = w * (W5 // C) + sub
                            cs = ds(sub * C, C)
                            psv = ppv.tile([128, GD], F32, tag="pv")
                            chains = ((xhw, wvh_sb), (xlw, wvh_sb), (xhw, wvl_sb))
                            n = len(chains) * KT
                            i = 0
                            for xt_, wt_ in chains:
                                for kt_i in range(KT):
                                    nc.tensor.matmul(psv[:], xt_[:, kt_i, cs],
                                                     wt_[:, kt_i, :],
                                                     start=(i == 0), stop=(i == n - 1))
                                    i += 1
                            psb = ppv.tile([128, HG], F32, tag="pv")
                            for kt_i in range(KT):
                                nc.tensor.matmul(psb[:], xhw[:, kt_i, cs], wb_sb[:, kt_i, :],
                                                 start=(kt_i == 0), stop=(kt_i == KT - 1))
                            nc.scalar.copy(v_sb[:, tt, :], psv[:])
                            nc.vector.tensor_tensor(blog[:, tt, :], psb[:], bbx_sb[:], ADD)
                        nc.vector.tensor_reduce(acc[:, ds(8 + w, 1)],
                                                v_sb[:, ds(w * (W5 // C), W5 // C), :],
                                                axis=mybir.AxisListType.XY, op=ADD,
                                                apply_absolute_value=True)
                    nc.scalar.activation(beta[:], blog[:],
                                         mybir.ActivationFunctionType.Sigmoid)
                    nc.vector.tensor_scalar(beta_h[:], beta[:], 0.5, None, MUL)

                    # ---- thr_v; ternarize v (sign blocks on Act, is_gt/is_lt
                    # blocks on Pool); fold beta; AV loop ----
                    thbv = thr_chain(8, 12, (ccv_in, ccv_out) if use_cc else None,
                                     ppv, "pv", "v" + sfx)

                    def fold_v(vt, winc, nwc, bsrc, eng, nm):
                        for par in range(2):   # even / odd heads
                            src = vt[:].rearrange("p (c h2 u d) -> p c h2 u d",
                                                  h2=2, u=2, d=HD)[:, :, :, par, :]
                            dstv = vbt[:, winc].rearrange(
                                "p c h (u d) -> p c h u d", u=2)[:, :, par::2, par, :]
                            bc = bsrc[:, winc, par::2, None].to_broadcast([128, nwc, 2, HD])
                            eng.tensor_tensor(dstv, src, bc, MUL)

                    nwc = W5 // GD
                    for wv in range(NCH // nwc):
                        winc = ds(wv * nwc, nwc)
                        vin = v_sb[:, winc, :]
                        vt = gl.tile([128, W5], F16, tag="g", name=f"vt{wv}{sfx}")
                        vtv = vt[:].rearrange("p (c m) -> p c m", m=GD)
                        if wv % 2 == 0:
                            b1 = gl.tile([128, W5], F16, tag="l", name=f"b1v{wv}{sfx}")
                            nc.scalar.activation(b1[:].rearrange("p (c m) -> p c m", m=GD),
                                                 vin, SIGN, bias=thbv[:, 1:2])
                            nc.scalar.activation(vtv, vin, SIGN, bias=thbv[:, 0:1])
                            nc.gpsimd.tensor_tensor(vt[:], vt[:], b1[:], ADD)
                            fold_v(vt, winc, nwc, beta_h, nc.vector, f"f{wv}{sfx}")
                        else:
                            g = gl.tile([128, W5], F32, tag="gg", name=f"g{wv}{sfx}")
                            l = gl.tile([128, W5], F32, tag="ll", name=f"l{wv}{sfx}")
                            nc.gpsimd.tensor_scalar(g[:].rearrange("p (c m) -> p c m", m=GD),
                                                    vin, thbv[:, 0:1], None,
                                                    mybir.AluOpType.is_gt)
                            nc.gpsimd.tensor_scalar(l[:].rearrange("p (c m) -> p c m", m=GD),
                                                    vin, thbv[:, 1:2], None,
                                                    mybir.AluOpType.is_lt)
                            nc.vector.tensor_tensor(vt[:], g[:], l[:], SUB)
                            fold_v(vt, winc, nwc, beta, nc.gpsimd, f"f{wv}{sfx}")

                    # AV: both band parts and both heads of a pair accumulate
                    # into one [128, C] PSUM image (zero-padded vbt)
                    for jc in range(NCH):
                        av = ppa.tile([128, 2, C], F32, tag="av", name=f"av_{jc}{sfx}",
                                      bufs=2)
                        for hp in range(2):
                            first = True
                            for hh in range(2):
                                h = hp * 2 + hh
                                slot = (h % 2) * 2 + h // 2
                                if jc > 0:
                                    nc.tensor.matmul(av[:, hp, :],
                                                     vbt[:, jc - 1, h, :],
                                                     sts_all[:, jc - 1, slot, ds(C, C)],
                                                     start=first, stop=False)
                                    first = False
                                nc.tensor.matmul(av[:, hp, :],
                                                 vbt[:, jc, h, :],
                                                 sts_all[:, jc, slot, 0:C],
                                                 start=first, stop=(hh == 1))
                                first = False
                        if jc % 2 == 0:
                            nc.scalar.copy(oT[:, :, ds(jc * C, C)], av[:])
                        else:
                            nc.vector.tensor_copy(oT[:, :, ds(jc * C, C)], av[:])

                # ---- output projection, PSUM -> SBUF(fp16) -> DRAM ----
                wo_sb = wpool.tile([128, 2, D_MODEL], F16, tag="wslot", name="wo" + sfx)
                nc.sync.dma_start(wo_sb[:], wo_d.rearrange("(t p) m -> p t m", p=128))
                with tc.tile_pool(name="ppf" + sfx, bufs=2, space="PSUM") as ppf:
                    for jc in range(NCH):
                        fo = gl.tile([128, D_MODEL], F16, tag="fo", name=f"fo{jc}{sfx}")
                        for nn in range(D_MODEL // 512):
                            pf = ppf.tile([128, 512], F32, tag="pf", name=f"pf{jc}_{nn}{sfx}")
                            for hp in range(2):
                                nc.tensor.matmul(pf[:], oT[:, hp, ds(jc * 128, 128)],
                                                 wo_sb[:, hp, ds(nn * 512, 512)],
                                                 start=(hp == 0), stop=(hp == 1))
                            if nn == 0:
                                nc.scalar.copy(fo[:, ds(nn * 512, 512)], pf[:])
                            else:
                                nc.vector.tensor_copy(fo[:, ds(nn * 512, 512)], pf[:])
                        nc.sync.dma_start(out_d[ds(jc * 128, 128), :], fo[:])

    nc.finalize()
    return nc


def _host_tables(T, alpha_log, heads):
    inv = (np.float32(1.0) /
           (np.float32(ROPE_BASE) ** (np.arange(0, HD, 2, dtype=np.float32) / np.float32(HD))))
    tpos = np.arange(T, dtype=np.float32)
    freqs = tpos[None, :] * inv[:, None]          # [32, T] fp32
    cos32 = np.cos(freqs).astype(np.float32)
    sin32 = np.sin(freqs).astype(np.float32)
    ct = np.empty((128, T), np.float32)
    st = np.empty((128, T), np.float32)
    for r in range(128):
        jj = r % HD
        idx = jj % 32
        ct[r] = cos32[idx]
        st[r] = (-sin32[idx]) if jj < 32 else sin32[idx]

    alpha = (1.0 / (1.0 + np.exp(-alpha_log.astype(np.float32)))).astype(np.float32)
    la = np.log(np.clip(alpha[:, 0], np.float32(1e-6), None)).astype(np.float32)
    need = 104.0 / np.abs(la).min()
    assert need <= 2 * C, f"decay band too wide for 2-chunk attention: {need}"
    jl = np.arange(C, dtype=np.float32)
    il = np.arange(2 * C, dtype=np.float32)
    diff = (il[None, :] - jl[:, None]).astype(np.float32)   # [128, 256]
    dt2 = np.zeros((128, HG, 2 * C), np.float32)
    slot_order = (0, 2, 1, 3)   # psum row-group pairing; see build_nc
    with np.errstate(over="ignore"):
        for s_, hh in enumerate(slot_order):
            m = np.exp(diff * la[heads[hh]], dtype=np.float32)
            m[diff < 0] = 0.0
            # k ternary is {-2,0,2} (sign+sign); fold the 1/2 into the table
            dt2[:, s_, :] = 0.5 * m
    return ct, st, dt2


def _split16(a):
    hi = a.astype(np.float16)
    lo = (a.astype(np.float32) - hi.astype(np.float32)).astype(np.float16)
    return np.ascontiguousarray(hi), np.ascontiguousarray(lo)


def make_in_maps(x, Wq, Wk, Wv, Wo, Wb, bb, alpha_log, T):
    maps = []
    for c in range(N_CORES):
        b, hg = c // 4, c % 4
        sl = slice(hg * GD, (hg + 1) * GD)
        heads = list(range(hg * HG, (hg + 1) * HG))
        xh, xl = _split16(np.ascontiguousarray(x[b].T).astype(np.float32))
        wkh, wkl = _split16(np.ascontiguousarray(Wk[sl].T))
        wvh, wvl = _split16(np.ascontiguousarray(Wv[sl].T))
        ct, st, dt2 = _host_tables(T, alpha_log, heads)
        maps.append({
            "xh": xh, "xl": xl,
            "wkh": wkh, "wkl": wkl,
            "wvh": wvh, "wvl": wvl,
            "wq": np.ascontiguousarray(Wq[sl].T).astype(np.float16),
            "wb": np.ascontiguousarray(Wb[heads].T).astype(np.float16),
            "bbx": np.tile(bb[heads][None, :], (128, 1)).astype(np.float32),
            "wo": np.ascontiguousarray(Wo[:, sl].T).astype(np.float16),
            "ct": ct, "st": st, "dt2": dt2,
        })
    return maps


def kernel(x, Wq, Wk, Wv, Wo, Wb, bb, alpha_log):
    x = np.asarray(x, dtype=np.float32)
    T = x.shape[1]
    key = (T, N_CORES, True, 1)
    if key not in _NC_CACHE:
        _NC_CACHE[key] = build_nc(T=T, n_cores=N_CORES, use_cc=True, repeat=1)
    nc = _NC_CACHE[key]
    maps = make_in_maps(x, np.asarray(Wq, np.float32), np.asarray(Wk, np.float32),
                        np.asarray(Wv, np.float32), np.asarray(Wo, np.float32),
                        np.asarray(Wb, np.float32), np.asarray(bb, np.float32),
                        np.asarray(alpha_log, np.float32), T)
    res = run_bass_kernel_spmd(nc, maps, list(range(N_CORES)))
    out = np.zeros((B, T, D_MODEL), np.float32)
    for c in range(N_CORES):
        out[c // 4] += res.results[c]["out"].astype(np.float32)
    return out


# revision 26
# speedup vs baseline: 1.1589x; 1.1589x over previous
"""Trainium2 Bass kernel for nn_DeltaRuleMemory (decayed causal linear attention
with RoPE, ternary-STE k/v quantization and beta key gating).

Sharding: 8 cores = batch (2) x head-groups (4 groups of 4 heads). Each core
computes its (b, head-group) slice end-to-end; the only cross-core exchange is
a pair of 1-float AllReduces for the global ternary-quantization thresholds.
Host sums the 4 per-head-group partial output projections per batch.

Algorithm note: decay alpha = sigmoid(alpha_log) < 0.5 for every head, so
exp(log_alpha * d) underflows to exactly 0.0f for d >= 128 (as it does in the
reference's T x T decay matrix). Attention is therefore computed exactly as a
banded product: each 128-query chunk attends to its own chunk (masked decay)
and the previous chunk only; the two contributions accumulate in PSUM.

Precision: the ternary threshold compare is sensitive to k/v values, so the
k/v projections run as a 3-chain fp16 hi/lo split (x = xh + xl, W = Wh + Wl;
k = xh@Wh + xl@Wh + xh@Wl, fp32 PSUM accumulation) which is fp32-accurate to
~2^-21 but runs at bf16 matmul speed. q/beta run a single fp16 hi chain and
the attention + output projection run in fp16 (~1e-4 relative, well below the
quantization-boundary noise).

Schedule: phase A1 projects k and q and derives thr_k, so ternarized k, the
score matmuls and the decay multiply all execute under the shadow of phase A3
(the v projection). Only thr_v, v-ternarize, AV and the output projection sit
on the tail. Ternarize uses sign(x-thr)+sign(x+thr) pairs on Activation
(values {-2,0,2}; dt2 and a halved beta carry the 1/2 compensations), with
half the v blocks on GpSimd via is_gt/is_lt to balance engines.
"""
import numpy as np
from contextlib import ExitStack

import concourse.bass as bass
import concourse.tile as tile
import concourse.mybir as mybir
from concourse import bacc
from concourse.bass import ds
from concourse.bass_utils import run_bass_kernel_spmd

F32 = mybir.dt.float32
F16 = mybir.dt.float16
MUL = mybir.AluOpType.mult
ADD = mybir.AluOpType.add
SUB = mybir.AluOpType.subtract
SIGN = mybir.ActivationFunctionType.Sign

B, D_MODEL, NH, HD = 2, 1024, 16, 64
INNER = NH * HD
N_CORES = 8
HG = 4              # heads per core
GD = HG * HD        # inner dims per core (256)
C = 128             # attention chunk
ROPE_BASE = 10000.0
THR_MIN, THR_MAX = 0.01, 10.0

_NC_CACHE = {}


def build_nc(T=2048, n_cores=N_CORES, use_cc=True, repeat=1):
    """Build the SPMD bass program (identical on every core)."""
    KT = D_MODEL // 128          # 8 contraction tiles
    NCH = T // C                 # chunks
    W5 = min(512, T)             # free-dim window for [*, T] processing
    NW = T // W5

    nc = bacc.Bacc("TRN2", target_bir_lowering=False, debug=False,
                   enable_asserts=True, num_devices=n_cores)

    xh_d = nc.dram_tensor("xh", [D_MODEL, T], F16, kind="ExternalInput").ap()
    xl_d = nc.dram_tensor("xl", [D_MODEL, T], F16, kind="ExternalInput").ap()
    wkh_d = nc.dram_tensor("wkh", [D_MODEL, GD], F16, kind="ExternalInput").ap()
    wkl_d = nc.dram_tensor("wkl", [D_MODEL, GD], F16, kind="ExternalInput").ap()
    wvh_d = nc.dram_tensor("wvh", [D_MODEL, GD], F16, kind="ExternalInput").ap()
    wvl_d = nc.dram_tensor("wvl", [D_MODEL, GD], F16, kind="ExternalInput").ap()
    wq_d = nc.dram_tensor("wq", [D_MODEL, GD], F16, kind="ExternalInput").ap()
    wb_d = nc.dram_tensor("wb", [D_MODEL, HG], F16, kind="ExternalInput").ap()
    bbx_d = nc.dram_tensor("bbx", [128, HG], F32, kind="ExternalInput").ap()
    wo_d = nc.dram_tensor("wo", [GD, D_MODEL], F16, kind="ExternalInput").ap()
    ct_d = nc.dram_tensor("ct", [128, T], F32, kind="ExternalInput").ap()
    st_d = nc.dram_tensor("st", [128, T], F32, kind="ExternalInput").ap()
    dt2_d = nc.dram_tensor("dt2", [128, HG, 2 * C], F32, kind="ExternalInput").ap()
    out_d = nc.dram_tensor("out", [T, D_MODEL], F16, kind="ExternalOutput").ap()

    if use_cc:
        cck_in = nc.dram_tensor("cck_in", [1, 1], F32)
        cck_out = nc.dram_tensor("cck_out", [1, 1], F32, addr_space="Shared")
        ccv_in = nc.dram_tensor("ccv_in", [1, 1], F32)
        ccv_out = nc.dram_tensor("ccv_out", [1, 1], F32, addr_space="Shared")

    n_elem = float(B * T * INNER) if use_cc else float(T * GD)

    with tile.TileContext(nc) as tc, ExitStack() as ctx:
        cpool = ctx.enter_context(tc.tile_pool(name="const", bufs=1))
        wpool = ctx.enter_context(tc.tile_pool(name="w", bufs=2))
        big = ctx.enter_context(tc.tile_pool(name="big", bufs=1))
        scr = ctx.enter_context(tc.tile_pool(name="scr", bufs=3))
        gl = ctx.enter_context(tc.tile_pool(name="gl", bufs=2))
        tiny = ctx.enter_context(tc.tile_pool(name="tiny", bufs=1))
        xpool = ctx.enter_context(tc.tile_pool(name="xs", bufs=2))

        # ---- weights for the k chain + first x window go first so the PE
        # starts early; everything else queues behind them ----
        wkh_sb = cpool.tile([128, KT, GD], F16, tag="wkh")
        nc.sync.dma_start(wkh_sb[:], wkh_d.rearrange("(ko p) m -> p ko m", p=128))
        wkl_sb = cpool.tile([128, KT, GD], F16, tag="wkl")
        nc.sync.dma_start(wkl_sb[:], wkl_d.rearrange("(ko p) m -> p ko m", p=128))

        xh_r = xh_d.rearrange("(ko p) t -> p ko t", p=128)
        xl_r = xl_d.rearrange("(ko p) t -> p ko t", p=128)

        def load_x(w, name_sfx, with_lo=True, nsplit=1):
            win = ds(w * W5, W5)
            xhw = xpool.tile([128, KT, W5], F16, tag="xh", name=f"xh{name_sfx}")
            xlw = xpool.tile([128, KT, W5], F16, tag="xl", name=f"xl{name_sfx}")
            step = KT // nsplit
            for s in range(nsplit):
                ks = ds(s * step, step)
                nc.sync.dma_start(xhw[:, ks, :], xh_r[:, ks, win])
            if with_lo:
                for s in range(nsplit):
                    ks = ds(s * step, step)
                    nc.sync.dma_start(xlw[:, ks, :], xl_r[:, ks, win])
            return xhw, xlw

        xw_pend = load_x(0, "_a0", nsplit=2)

        wq_sb = cpool.tile([128, KT, GD], F16, tag="wq")
        nc.sync.dma_start(wq_sb[:], wq_d.rearrange("(ko p) m -> p ko m", p=128))
        ct_sb = cpool.tile([128, T], F32, tag="ct")
        st_sb = cpool.tile([128, T], F32, tag="st")
        nc.sync.dma_start(ct_sb[:], ct_d[:])
        nc.sync.dma_start(st_sb[:], st_d[:])
        dt2_sb = cpool.tile([128, HG, 2 * C], F32, tag="dt2")
        nc.sync.dma_start(dt2_sb[:], dt2_d[:])
        wvh_sb = cpool.tile([128, KT, GD], F16, tag="wvh")
        nc.sync.dma_start(wvh_sb[:], wvh_d.rearrange("(ko p) m -> p ko m", p=128))
        wvl_sb = cpool.tile([128, KT, GD], F16, tag="wvl")
        nc.sync.dma_start(wvl_sb[:], wvl_d.rearrange("(ko p) m -> p ko m", p=128))
        wb_sb = cpool.tile([128, KT, HG], F16, tag="wb")
        nc.sync.dma_start(wb_sb[:], wb_d.rearrange("(ko p) h -> p ko h", p=128))
        bbx_sb = cpool.tile([128, HG], F32, tag="bbx")
        nc.sync.dma_start(bbx_sb[:], bbx_d[:])

        for rep in range(repeat):
            sfx = f"_r{rep}" if repeat > 1 else ""

            kT = [big.tile([128, T], F32, tag=f"kT{i}", name=f"kT{i}{sfx}") for i in range(2)]
            kTt = [big.tile([128, T], F16, tag=f"kTt{i}", name=f"kTt{i}{sfx}") for i in range(2)]
            qT = [big.tile([128, T], F16, tag=f"qT{i}", name=f"qT{i}{sfx}") for i in range(2)]
            v_sb = big.tile([128, NCH, GD], F32, tag="v", name=f"v{sfx}")
            # zero-padded ternary v: head h occupies columns (h%2)*64..+64 of
            # slot [jc, h]; the other half stays 0 so AV matmuls write the full
            # 128-row PSUM with head pairs at partition offsets 0/64.
            vbt = big.tile([128, NCH, HG, 2 * HD], F16, tag="vbt", name=f"vbt{sfx}")
            blog = big.tile([128, NCH, HG], F32, tag="blog", name=f"blog{sfx}")
            beta = big.tile([128, NCH, HG], F16, tag="beta", name=f"beta{sfx}")
            beta_h = big.tile([128, NCH, HG], F16, tag="beta_h", name=f"beta_h{sfx}")
            oT = big.tile([128, 2, T], F16, tag="oT", name=f"oT{sfx}")
            sts_all = big.tile([128, NCH, HG, 2 * C], F16, tag="sts", name=f"sts{sfx}")
            acc = tiny.tile([128, 16], F32, tag="acc", name="acc" + sfx)

            # zero the dead halves of vbt once, off the critical path
            vbt4 = vbt[:].rearrange("p c h (u d) -> p c h u d", u=2)
            nc.vector.memset(vbt4[:, :, 0::2, 1, :], 0.0)
            nc.vector.memset(vbt4[:, :, 1::2, 0, :], 0.0)

            def rope(ps, dst, win, nm):
                """dst[:, win] = rope(ps); rotate copies split Act/Pool."""
                rot = scr.tile([128, W5], F32, tag="rot", name=f"rot{nm}")
                for hb in range(2):
                    nc.scalar.copy(rot[ds(hb * 64, 32), :], ps[ds(hb * 64 + 32, 32), :])
                    nc.scalar.copy(rot[ds(hb * 64 + 32, 32), :], ps[ds(hb * 64, 32), :])
                nc.vector.tensor_tensor(rot[:], rot[:], st_sb[:, win], MUL)
                nc.vector.tensor_tensor(dst[:, win], ps[:], ct_sb[:, win], MUL)
                nc.vector.tensor_tensor(dst[:, win], dst[:, win], rot[:], ADD)

            def thr_chain(col_lo, col_hi, cc_pair, pool, tag, nm, tbufs=1):
                """acc cols [col_lo, col_hi) -> broadcast [128,2] (thr, -thr)."""
                onesP = tiny.tile([128, 1], F32, tag="onesP", name=f"onesP{nm}")
                nc.vector.memset(onesP[:], 1.0)
                pstt = pool.tile([1, col_hi - col_lo], F32, tag=tag, name=f"pst{nm}",
                                 bufs=tbufs)
                nc.tensor.matmul(pstt[:], onesP[:], acc[:, ds(col_lo, col_hi - col_lo)],
                                 start=True, stop=True)
                sct = tiny.tile([1, 1], F32, tag=f"sc{nm}", name=f"sc{nm}")
                nc.vector.tensor_reduce(sct[:], pstt[0:1, :],
                                        axis=mybir.AxisListType.X, op=ADD)
                if cc_pair is not None:
                    cin, cout = cc_pair
                    nc.sync.dma_start(cin[:], sct[:])
                    nc.gpsimd.collective_compute(
                        "AllReduce", ADD,
                        replica_groups=[list(range(n_cores))],
                        ins=[cin[:]], outs=[cout[:]])
                    tott = tiny.tile([1, 1], F32, tag=f"tot{nm}", name=f"tot{nm}")
                    nc.sync.dma_start(tott[:], cout[:])
                else:
                    tott = sct
                th = tiny.tile([1, 2], F32, tag=f"th{nm}", name=f"th{nm}")
                nc.vector.tensor_scalar(th[0:1, 0:1], tott[0:1, :], 1.0 / n_elem, None, MUL)
                nc.vector.tensor_scalar(th[0:1, 0:1], th[0:1, 0:1], THR_MIN, THR_MAX,
                                        mybir.AluOpType.max, mybir.AluOpType.min)
                nc.vector.tensor_scalar(th[0:1, 1:2], th[0:1, 0:1], -1.0, None, MUL)
                ones1 = tiny.tile([1, 128], F32, tag="ones1", name=f"ones1{nm}")
                nc.vector.memset(ones1[:], 1.0)
                psb2 = pool.tile([128, 2], F32, tag=tag, name=f"psb2{nm}", bufs=tbufs)
                nc.tensor.matmul(psb2[:], ones1[:], th[:], start=True, stop=True)
                thb = tiny.tile([128, 2], F32, tag=f"thb{nm}", name=f"thb{nm}")
                nc.vector.tensor_copy(thb[:], psb2[:])   # [thr, -thr]
                return thb

            with tc.tile_pool(name="ppa" + sfx, bufs=1, space="PSUM") as ppa:
                # ---- phase A1: k (3-chain) + q (hi chain) projections ----
                with tc.tile_pool(name="ppq" + sfx, bufs=3, space="PSUM") as ppq:
                    for w in range(NW):
                        win = ds(w * W5, W5)
                        xhw, xlw = xw_pend
                        if w + 1 < NW:
                            xw_pend = load_x(w + 1, f"_a{w + 1}{sfx}")
                        for mt in range(2):
                            ps = ppq.tile([128, W5], F32, tag="proj")
                            chains = ((wkh_sb, xhw), (wkh_sb, xlw), (wkl_sb, xhw))
                            n = len(chains) * KT
                            i = 0
                            for wt_, xt_ in chains:
                                for kt_i in range(KT):
                                    nc.tensor.matmul(ps[:], wt_[:, kt_i, ds(mt * 128, 128)],
                                                     xt_[:, kt_i, :],
                                                     start=(i == 0), stop=(i == n - 1))
                                    i += 1
                            rope(ps, kT[mt], win, f"k{mt}_{w}{sfx}")
                            nc.vector.tensor_reduce(acc[:, ds(w * 2 + mt, 1)], kT[mt][:, win],
                                                    axis=mybir.AxisListType.X, op=ADD,
                                                    apply_absolute_value=True)
                        for mt in range(2):
                            psq = ppq.tile([128, W5], F32, tag="proj")
                            for kt_i in range(KT):
                                nc.tensor.matmul(psq[:], wq_sb[:, kt_i, ds(mt * 128, 128)],
                                                 xhw[:, kt_i, :],
                                                 start=(kt_i == 0), stop=(kt_i == KT - 1))
                            rope(psq, qT[mt], win, f"q{mt}_{w}{sfx}")

                    # ---- thr_k; ternarize k (scores issue inside phase A3) ----
                    thbk = thr_chain(0, 8, (cck_in, cck_out) if use_cc else None,
                                     ppq, "pt", "k" + sfx)
                    for w in range(NW):
                        win = ds(w * W5, W5)
                        for mt in range(2):
                            a1 = scr.tile([128, W5], F16, tag="rot", name=f"a1k{mt}_{w}{sfx}")
                            a2 = scr.tile([128, W5], F16, tag="rot", name=f"a2k{mt}_{w}{sfx}")
                            nc.scalar.activation(a1[:], kT[mt][:, win], SIGN, bias=thbk[:, 1:2])
                            nc.scalar.activation(a2[:], kT[mt][:, win], SIGN, bias=thbk[:, 0:1])
                            nc.vector.tensor_tensor(kTt[mt][:, win], a1[:], a2[:], ADD)

                def scores(jc):
                    ilen = min(2 * C, T - jc * C)
                    for grp in range(2):
                        spg = ppa.tile([128, 2, 2 * C], F32, tag=f"s{grp}",
                                       name=f"s{grp}_{jc}{sfx}")
                        for j, h in enumerate((grp, grp + 2)):
                            tl, po = h // 2, (h % 2) * 64
                            nc.tensor.matmul(
                                spg[:, j, 0:ilen],
                                kTt[tl][ds(po, 64), ds(jc * C, C)],
                                qT[tl][ds(po, 64), ds(jc * C, ilen)],
                                start=True, stop=True)
                        nc.vector.tensor_tensor(
                            sts_all[:, jc, ds(grp * 2, 2), 0:ilen],
                            spg[:, :, 0:ilen],
                            dt2_sb[:, ds(grp * 2, 2), 0:ilen], MUL)

                # ---- phase A3: v (3-chain) + beta projections ----
                with tc.tile_pool(name="ppv" + sfx, bufs=2, space="PSUM") as ppv:
                    xw_pend = load_x(0, "_b0" + sfx)
                    for w in range(NW):
                        xhw, xlw = xw_pend
                        if w + 1 < NW:
                            xw_pend = load_x(w + 1, f"_b{w + 1}{sfx}")
                        for sub in range(W5 // C):
                            tt = w * (W5 // C) + sub
                            cs = ds(sub * C, C)
                            psv = ppv.tile([128, GD], F32, tag="pv")
                            chains = ((xhw, wvh_sb), (xlw, wvh_sb), (xhw, wvl_sb))
                            n = len(chains) * KT
                            i = 0
                            for xt_, wt_ in chains:
                                for kt_i in range(KT):
                                    nc.tensor.matmul(psv[:], xt_[:, kt_i, cs],
                                                     wt_[:, kt_i, :],
                                                     start=(i == 0), stop=(i == n - 1))
                                    i += 1
                            psb = ppv.tile([128, HG], F32, tag="pv")
                            for kt_i in range(KT):
                                nc.tensor.matmul(psb[:], xhw[:, kt_i, cs], wb_sb[:, kt_i, :],
                                                 start=(kt_i == 0), stop=(kt_i == KT - 1))
                            nc.scalar.copy(v_sb[:, tt, :], psv[:])
                            nc.vector.tensor_tensor(blog[:, tt, :], psb[:], bbx_sb[:], ADD)
                        nc.vector.tensor_reduce(acc[:, ds(8 + w, 1)],
                                                v_sb[:, ds(w * (W5 // C), W5 // C), :],
                                                axis=mybir.AxisListType.XY, op=ADD,
                                                apply_absolute_value=True)
                        for jc in range(w * 4, w * 4 + 4):
                            scores(jc)
                    nc.scalar.activation(beta[:], blog[:],
                                         mybir.ActivationFunctionType.Sigmoid)
                    nc.vector.tensor_scalar(beta_h[:], beta[:], 0.5, None, MUL)

                    # ---- thr_v; ternarize v (sign blocks on Act, is_gt/is_lt
                    # blocks on Pool); fold beta; AV loop ----
                    thbv = thr_chain(8, 12, (ccv_in, ccv_out) if use_cc else None,
                                     ppv, "pv", "v" + sfx, tbufs=2)

                    def fold_v(vt, winc, nwc, bsrc, eng, nm):
                        for par in range(2):   # even / odd heads
                            src = vt[:].rearrange("p (c h2 u d) -> p c h2 u d",
                                                  h2=2, u=2, d=HD)[:, :, :, par, :]
                            dstv = vbt[:, winc].rearrange(
                                "p c h (u d) -> p c h u d", u=2)[:, :, par::2, par, :]
                            bc = bsrc[:, winc, par::2, None].to_broadcast([128, nwc, 2, HD])
                            eng.tensor_tensor(dstv, src, bc, MUL)

                    nwc = W5 // GD
                    for wv in range(NCH // nwc):
                        winc = ds(wv * nwc, nwc)
                        vin = v_sb[:, winc, :]
                        vt = gl.tile([128, W5], F16, tag="g", name=f"vt{wv}{sfx}")
                        vtv = vt[:].rearrange("p (c m) -> p c m", m=GD)
                        if wv % 2 == 0:
                            b1 = gl.tile([128, W5], F16, tag="l", name=f"b1v{wv}{sfx}")
                            nc.scalar.activation(b1[:].rearrange("p (c m) -> p c m", m=GD),
                                                 vin, SIGN, bias=thbv[:, 1:2])
                            nc.scalar.activation(vtv, vin, SIGN, bias=thbv[:, 0:1])
                            nc.gpsimd.tensor_tensor(vt[:], vt[:], b1[:], ADD)
                            fold_v(vt, winc, nwc, beta_h, nc.vector, f"f{wv}{sfx}")
                        else:
                            g = gl.tile([128, W5], F32, tag="gg", name=f"g{wv}{sfx}")
                            l = gl.tile([128, W5], F32, tag="ll", name=f"l{wv}{sfx}")
                            nc.gpsimd.tensor_scalar(g[:].rearrange("p (c m) -> p c m", m=GD),
                                                    vin, thbv[:, 0:1], None,
                                                    mybir.AluOpType.is_gt)
                            nc.gpsimd.tensor_scalar(l[:].rearrange("p (c m) -> p c m", m=GD),
                                                    vin, thbv[:, 1:2], None,
                                                    mybir.AluOpType.is_lt)
                            nc.vector.tensor_tensor(vt[:], g[:], l[:], SUB)
                            fold_v(vt, winc, nwc, beta, nc.gpsimd, f"f{wv}{sfx}")

                    # AV: both band parts and both heads of a pair accumulate
                    # into one [128, C] PSUM image (zero-padded vbt)
                    for jc in range(NCH):
                        av = ppa.tile([128, 2, C], F32, tag="av", name=f"av_{jc}{sfx}",
                                      bufs=2)
                        for hp in range(2):
                            first = True
                            for hh in range(2):
                                h = hp * 2 + hh
                                slot = (h % 2) * 2 + h // 2
                                if jc > 0:
                                    nc.tensor.matmul(av[:, hp, :],
                                                     vbt[:, jc - 1, h, :],
                                                     sts_all[:, jc - 1, slot, ds(C, C)],
                                                     start=first, stop=False)
                                    first = False
                                nc.tensor.matmul(av[:, hp, :],
                                                 vbt[:, jc, h, :],
                                                 sts_all[:, jc, slot, 0:C],
                                                 start=first, stop=(hh == 1))
                                first = False
                        if jc % 2 == 0:
                            nc.scalar.copy(oT[:, :, ds(jc * C, C)], av[:])
                        else:
                            nc.vector.tensor_copy(oT[:, :, ds(jc * C, C)], av[:])

                # ---- output projection, PSUM -> SBUF(fp16) -> DRAM ----
                wo_sb = wpool.tile([128, 2, D_MODEL], F16, tag="wslot", name="wo" + sfx)
                nc.sync.dma_start(wo_sb[:], wo_d.rearrange("(t p) m -> p t m", p=128))
                with tc.tile_pool(name="ppf" + sfx, bufs=2, space="PSUM") as ppf:
                    for jc in range(NCH):
                        fo = gl.tile([128, D_MODEL], F16, tag="fo", name=f"fo{jc}{sfx}")
                        for nn in range(D_MODEL // 512):
                            pf = ppf.tile([128, 512], F32, tag="pf", name=f"pf{jc}_{nn}{sfx}")
                            for hp in range(2):
                                nc.tensor.matmul(pf[:], oT[:, hp, ds(jc * 128, 128)],
                                                 wo_sb[:, hp, ds(nn * 512, 512)],
                                                 start=(hp == 0), stop=(hp == 1))
                            if nn == 0:
                                nc.scalar.copy(fo[:, ds(nn * 512, 512)], pf[:])
                            else:
                                nc.vector.tensor_copy(fo[:, ds(nn * 512, 512)], pf[:])
                        nc.sync.dma_start(out_d[ds(jc * 128, 128), :], fo[:])

    nc.finalize()
    return nc


def _host_tables(T, alpha_log, heads):
    inv = (np.float32(1.0) /
           (np.float32(ROPE_BASE) ** (np.arange(0, HD, 2, dtype=np.float32) / np.float32(HD))))
    tpos = np.arange(T, dtype=np.float32)
    freqs = tpos[None, :] * inv[:, None]          # [32, T] fp32
    cos32 = np.cos(freqs).astype(np.float32)
    sin32 = np.sin(freqs).astype(np.float32)
    ct = np.empty((128, T), np.float32)
    st = np.empty((128, T), np.float32)
    for r in range(128):
        jj = r % HD
        idx = jj % 32
        ct[r] = cos32[idx]
        st[r] = (-sin32[idx]) if jj < 32 else sin32[idx]

    alpha = (1.0 / (1.0 + np.exp(-alpha_log.astype(np.float32)))).astype(np.float32)
    la = np.log(np.clip(alpha[:, 0], np.float32(1e-6), None)).astype(np.float32)
    need = 104.0 / np.abs(la).min()
    assert need <= 2 * C, f"decay band too wide for 2-chunk attention: {need}"
    jl = np.arange(C, dtype=np.float32)
    il = np.arange(2 * C, dtype=np.float32)
    diff = (il[None, :] - jl[:, None]).astype(np.float32)   # [128, 256]
    dt2 = np.zeros((128, HG, 2 * C), np.float32)
    slot_order = (0, 2, 1, 3)   # psum row-group pairing; see build_nc
    with np.errstate(over="ignore"):
        for s_, hh in enumerate(slot_order):
            m = np.exp(diff * la[heads[hh]], dtype=np.float32)
            m[diff < 0] = 0.0
            # k ternary is {-2,0,2} (sign+sign); fold the 1/2 into the table
            dt2[:, s_, :] = 0.5 * m
    return ct, st, dt2


def _split16(a):
    hi = a.astype(np.float16)
    lo = (a.astype(np.float32) - hi.astype(np.float32)).astype(np.float16)
    return np.ascontiguousarray(hi), np.ascontiguousarray(lo)


def make_in_maps(x, Wq, Wk, Wv, Wo, Wb, bb, alpha_log, T):
    maps = []
    for c in range(N_CORES):
        b, hg = c // 4, c % 4
        sl = slice(hg * GD, (hg + 1) * GD)
        heads = list(range(hg * HG, (hg + 1) * HG))
        xh, xl = _split16(np.ascontiguousarray(x[b].T).astype(np.float32))
        wkh, wkl = _split16(np.ascontiguousarray(Wk[sl].T))
        wvh, wvl = _split16(np.ascontiguousarray(Wv[sl].T))
        ct, st, dt2 = _host_tables(T, alpha_log, heads)
        maps.append({
            "xh": xh, "xl": xl,
            "wkh": wkh, "wkl": wkl,
            "wvh": wvh, "wvl": wvl,
            "wq": np.ascontiguousarray(Wq[sl].T).astype(np.float16),
            "wb": np.ascontiguousarray(Wb[heads].T).astype(np.float16),
            "bbx": np.tile(bb[heads][None, :], (128, 1)).astype(np.float32),
            "wo": np.ascontiguousarray(Wo[:, sl].T).astype(np.float16),
            "ct": ct, "st": st, "dt2": dt2,
        })
    return maps


def kernel(x, Wq, Wk, Wv, Wo, Wb, bb, alpha_log):
    x = np.asarray(x, dtype=np.float32)
    T = x.shape[1]
    key = (T, N_CORES, True, 1)
    if key not in _NC_CACHE:
        _NC_CACHE[key] = build_nc(T=T, n_cores=N_CORES, use_cc=True, repeat=1)
    nc = _NC_CACHE[key]
    maps = make_in_maps(x, np.asarray(Wq, np.float32), np.asarray(Wk, np.float32),
                        np.asarray(Wv, np.float32), np.asarray(Wo, np.float32),
                        np.asarray(Wb, np.float32), np.asarray(bb, np.float32),
                        np.asarray(alpha_log, np.float32), T)
    res = run_bass_kernel_spmd(nc, maps, list(range(N_CORES)))
    out = np.zeros((B, T, D_MODEL), np.float32)
    for c in range(N_CORES):
        out[c // 4] += res.results[c]["out"].astype(np.float32)
    return out
